# revision 40
# baseline (speedup 1.0000x reference)
"""Trainium2 Bass kernel for nn_ConvSPE (depthwise-conv SPE + per-channel contraction).

Math (reference): per bn=(b,nu) row and channel d:
    pe_k = noise / sqrt(num*d)                       (b*num, d, s+2k)
    pe_q = depthwise_valid_xcorr(pe_k, w)            k=200 taps, same filter per channel
    qhat[b,nu,t] = sum_d pe_q[bn,d,t]      * q[b,d,t]
    khat[b,nu,t] = sum_d pe_k[bn,d,t+k//2] * k[b,d,t]

Kernel strategy (8 NeuronCores, data-parallel over the 128 bn rows; 16 rows/core,
grouped into 8 pairs; all rows of a core share one b):

  * Swapped-operand conv: stationary = x window-pairs
    xf[p, n, r2, d] = x[2*pair+r2][d, 128n+p]  ([128 time, 128=(r2,d)] per window),
    moving = 3 shared Toeplitz chunks T_s[p, tau] = w[128s+p-tau].  Output block
    psq[(r2,d), tau] accumulates in PSUM; the third chunk only has support for
    tau >= 57, and its moving operand is f-sliced accordingly, so a 128-wide
    output block costs 327 (not 384) PE cycles.  Conv output lands with the
    channel pack (r2,d) on PARTITIONS.
  * qhat: ACT copies psq -> SBUF fp16, DVE multiplies by queries in the same
    (r2,d)-partition layout (fp16 2x), then per-128 time chunk one tiny matmul
    (stationary = product chunk, moving = [128,2] half-selector) contracts the
    partition dim: out[tau, r2] = sum_d pq[(r2,d), tau].  Out free = 2.
  * khat needs no conv.  Last N_PAR rows: host ships x in parity layout
    xk[64*par+d, u] = x[d, 2u+par+100]; DVE multiplies by matching keys and the
    same selector-matmul contracts d.  First rows: DVE multiplies conv-layout x
    windows by shifted/scaled keys; an fp16 add-tree over d runs on Pool
    (first POOL_TPAIRS pairs) and DVE (rest) to balance engine load.
  * DMA order is hand-scheduled (the model serializes DMA), PE is software-
    pipelined at conv-half granularity, and output quarters drain early so the
    tail after the last conv is short.
"""

import math
import numpy as np

_CACHE = {}


def _ensure_paths():
    try:
        import concourse  # noqa: F401
    except ImportError:
        import sys

        for p in ("/opt/trn_rl_repo", "/root/.axon_site/_ro/trn_rl_repo"):
            if p not in sys.path:
                sys.path.insert(0, p)


N_CORES = 8
B, D, L, K, NUM = 4, 64, 4096, 200, 32
NW = 34  # x windows of 128 per row (covers t+j up to 4351)
NT = 32  # output time blocks of 128
ROWS = 16  # bn rows per core
PAIRS = ROWS // 2
N_PAR = 8  # rows using parity-layout khat (the last N_PAR rows of the core)
TPAIRS = (ROWS - N_PAR) // 2  # pairs using conv-layout khat + add-tree
POOL_TPAIRS = 4  # tree pairs whose add-tree runs on Pool (rest on DVE)
NKT = 33  # khat tree n-window count
UC = 16  # parity khat 128-chunks per row (2048 u's)
F3 = 71  # third conv chunk: tau in [57,128), contraction over 71 partitions
S3 = 57


def build_module():
    """Build + compile the per-core Bass module (identical SPMD program)."""
    _ensure_paths()
    from contextlib import ExitStack

    import concourse.bacc as bacc
    import concourse.mybir as mybir
    import concourse.tile as tile

    F16 = mybir.dt.float16
    F32 = mybir.dt.float32

    nc = bacc.Bacc(
        "TRN2", target_bir_lowering=False, debug=False, num_devices=N_CORES
    )

    xf_d = nc.dram_tensor("xf", [PAIRS, 128, NW, 2, D], F16, kind="ExternalInput").ap()
    cst_d = nc.dram_tensor("cst", [128, 3 * 128 + 2], F16, kind="ExternalInput").ap()
    qt_d = nc.dram_tensor("qt", [128, NT, 128], F16, kind="ExternalInput").ap()
    kf_d = nc.dram_tensor("kf", [128, NKT, D], F16, kind="ExternalInput").ap()
    xk_d = nc.dram_tensor("xk", [N_PAR, 128, UC, 128], F16, kind="ExternalInput").ap()
    kk_d = nc.dram_tensor("kk", [128, UC, 128], F16, kind="ExternalInput").ap()
    qo_d = nc.dram_tensor("qo", [128, PAIRS, NT, 2], F32, kind="ExternalOutput").ap()
    ko_d = nc.dram_tensor("ko", [128, N_PAR, UC, 2], F32, kind="ExternalOutput").ap()
    kt_d = nc.dram_tensor("kt", [128, TPAIRS, NKT, 2], F32, kind="ExternalOutput").ap()

    with tile.TileContext(nc) as tc, ExitStack() as ctx:
        wp = ctx.enter_context(tc.tile_pool(name="const", bufs=1))
        xp = ctx.enter_context(tc.tile_pool(name="x", bufs=5))
        xkpl = ctx.enter_context(tc.tile_pool(name="xk", bufs=N_PAR))
        pp = ctx.enter_context(tc.tile_pool(name="psum", bufs=3, space="PSUM"))
        qpp = ctx.enter_context(tc.tile_pool(name="qps", bufs=1, space="PSUM"))
        kpp = ctx.enter_context(tc.tile_pool(name="kps", bufs=1, space="PSUM"))
        cp = ctx.enter_context(tc.tile_pool(name="peq", bufs=3))
        qp = ctx.enter_context(tc.tile_pool(name="prodq", bufs=3))
        kp = ctx.enter_context(tc.tile_pool(name="prodk", bufs=4))
        tp = ctx.enter_context(tc.tile_pool(name="tree", bufs=2))
        op = ctx.enter_context(tc.tile_pool(name="out", bufs=1))

        cst = wp.tile([128, 3 * 128 + 2], F16, tag="cst")
        nc.sync.dma_start(cst[:], cst_d[:])
        mv = [cst[:, 128 * s : 128 * (s + 1)] for s in range(3)]
        sel_t = cst[:, 384:386]

        qps = qpp.tile([128, PAIRS, NT, 2], F32, tag="qps")
        kps = kpp.tile([128, N_PAR, UC, 2], F32, tag="kps")
        kacc = op.tile([128, TPAIRS, NKT, 2], F32, tag="ka")
        qos = op.tile([128, PAIRS, NT, 2], F32, tag="qos")
        kos = op.tile([128, N_PAR, UC, 2], F32, tag="kos")

        # Warm the PE p-state with throwaway matmuls while the first conv
        # input streams in (full clock needs ~3us of continuous PE activity).
        # Zeros tile needs no DMA; they scribble into qps, which every
        # selector matmul later overwrites.
        wz = wp.tile([128, 128], F16, tag="warmz")
        nc.vector.memset(wz[:], 0.0)
        for i in range(26):
            nc.tensor.matmul(
                qps[:, 2 * (i % 4) : 2 * (i % 4) + 2, :, :], wz[:], wz[:],
                start=True, stop=True,
            )

        # DMA order is execution order on the (serialized) DMA engines: first
        # conv input first, keys for the Pool trees early, the last conv input
        # ahead of the final parity rows so the tail is cheap parity work.
        xf_tiles = {}

        def load_xf(pair, split=False):
            xf = xp.tile([128, NW, 2, D], F16, tag="xf", name=f"xf_{pair}")
            if split:
                nc.sync.dma_start(xf[:, 0:18, :, :], xf_d[pair, :, 0:18, :, :])
                nc.sync.dma_start(xf[:, 18:NW, :, :], xf_d[pair, :, 18:NW, :, :])
            else:
                nc.sync.dma_start(xf[:], xf_d[pair])
            xf_tiles[pair] = xf

        xk_tiles = {}

        def load_xk(r):
            xk = xkpl.tile([128, UC, 128], F16, tag="xk", name=f"xk_{r}")
            nc.sync.dma_start(xk[:], xk_d[r])
            xk_tiles[r] = xk

        kf_t = wp.tile([128, NKT, D], F16, tag="kf")
        kk_t = wp.tile([128, UC, 128], F16, tag="kk")
        qt_t = wp.tile([128, NT, 128], F16, tag="qt")
        xf0 = xp.tile([128, NW, 2, D], F16, tag="xf", name="xf_0")
        xf_tiles[0] = xf0
        nc.sync.dma_start(xf0[:, 0:18, :, :], xf_d[0, :, 0:18, :, :])
        nc.sync.dma_start(kf_t[:], kf_d[:])
        nc.sync.dma_start(xf0[:, 18:NW, :, :], xf_d[0, :, 18:NW, :, :])
        load_xf(1)

        pk_par = {}
        pkt_tiles = {}
        tree_tmp = {}

        def parity_mul(r):
            pk = kp.tile([128, UC, 128], F16, tag="pkp", name=f"pkp_{r}")
            nc.vector.tensor_mul(pk[:], xk_tiles[r][:], kk_t[:])
            pk_par[r] = pk

        def parity_sel(r):
            for c in range(UC):
                nc.tensor.matmul(
                    kps[:, r, c, :], pk_par[r][:, c, :], sel_t,
                    start=True, stop=True,
                )

        def tree_khat(pair, n0=0, n1=NKT):
            pkt = pkt_tiles.get(pair)
            if pkt is None:
                pkt = kp.tile([128, NKT, 2, D], F16, tag="pkt", name=f"pkt_{pair}")
                pkt_tiles[pair] = pkt
                ta = tp.tile([128, NKT, 2, 32], F16, tag="trA", name=f"trA_{pair}")
                tb = tp.tile([128, NKT, 2, 16], F16, tag="trB", name=f"trB_{pair}")
                tree_tmp[pair] = (ta, tb)
            ta, tb = tree_tmp[pair]
            s = slice(n0, n1)
            nc.vector.tensor_mul(
                pkt[:, s, 0, :], xf_tiles[pair][:, n0:n1, 0, :], kf_t[:, s, :]
            )
            nc.vector.tensor_mul(
                pkt[:, s, 1, :], xf_tiles[pair][:, n0:n1, 1, :], kf_t[:, s, :]
            )
            eng = nc.gpsimd if pair < POOL_TPAIRS else nc.vector
            a = ta[:, s, :, :]
            b = tb[:, s, :, :]
            pk = pkt[:, s, :, :]
            eng.tensor_add(a[:, :, :, 0:32], pk[:, :, :, 0:32], pk[:, :, :, 32:64])
            eng.tensor_add(b[:, :, :, 0:16], a[:, :, :, 0:16], a[:, :, :, 16:32])
            eng.tensor_add(a[:, :, :, 0:8], b[:, :, :, 0:8], b[:, :, :, 8:16])
            eng.tensor_add(b[:, :, :, 0:4], a[:, :, :, 0:4], a[:, :, :, 4:8])
            eng.tensor_add(a[:, :, :, 8:10], b[:, :, :, 0:2], b[:, :, :, 2:4])
            eng.tensor_add(kacc[:, pair, s, :], a[:, :, :, 8], a[:, :, :, 9])

        psq_last = {}

        def conv_half(pair, peq, h2, keep_psum=False, copy_pool=False):
            # swapped conv: 8 blocks per PSUM tile (2 banks), 2 tiles per half
            xf = xf_tiles[pair]
            for h in (2 * h2, 2 * h2 + 1):
                psq = pp.tile([128, 8, 128], F32, tag="psq", name=f"psq_{pair}_{h}")
                for b8 in range(8):
                    blk = h * 8 + b8
                    w0 = xf[:, blk, :, :]
                    w1 = xf[:, blk + 1, :, :]
                    w2 = xf[0:F3, blk + 2, :, :]
                    nc.tensor.matmul(
                        psq[:, b8, 0:S3], w0, mv[0][:, 0:S3], start=True, stop=False
                    )
                    nc.tensor.matmul(
                        psq[:, b8, 0:S3], w1, mv[1][:, 0:S3], start=False, stop=True
                    )
                    nc.tensor.matmul(
                        psq[:, b8, S3:128], w0, mv[0][:, S3:128],
                        start=True, stop=False,
                    )
                    nc.tensor.matmul(
                        psq[:, b8, S3:128], w1, mv[1][:, S3:128],
                        start=False, stop=False,
                    )
                    nc.tensor.matmul(
                        psq[:, b8, S3:128], w2, mv[2][0:F3, S3:128],
                        start=False, stop=True,
                    )
                if keep_psum:
                    psq_last[(pair, h)] = psq
                elif copy_pool:
                    nc.gpsimd.tensor_copy(peq[:, h * 8 : (h + 1) * 8, :], psq[:])
                else:
                    nc.scalar.copy(peq[:, h * 8 : (h + 1) * 8, :], psq[:])

        pq_tiles = {}

        def qhat_mul(pair, peq, h2):
            if pair not in pq_tiles:
                pq_tiles[pair] = qp.tile(
                    [128, NT, 128], F16, tag="pq", name=f"pq_{pair}"
                )
            sl = slice(h2 * 16, (h2 + 1) * 16)
            nc.vector.tensor_mul(
                pq_tiles[pair][:, sl, :], peq[:, sl, :], qt_t[:, sl, :]
            )

        def qhat_mul_direct(pair, h2):
            # read conv PSUM directly (fp32, no 2x) — used on the final pair
            # to cut the ACT-copy latency out of the tail
            if pair not in pq_tiles:
                pq_tiles[pair] = qp.tile(
                    [128, NT, 128], F16, tag="pq", name=f"pq_{pair}"
                )
            for h in (2 * h2, 2 * h2 + 1):
                sl = slice(h * 8, (h + 1) * 8)
                nc.vector.tensor_mul(
                    pq_tiles[pair][:, sl, :], psq_last[(pair, h)][:],
                    qt_t[:, sl, :],
                )

        def qhat_sel(pair, h2):
            for c in range(h2 * 16, (h2 + 1) * 16):
                nc.tensor.matmul(
                    qps[:, pair, c, :], pq_tiles[pair][:, c, :], sel_t,
                    start=True, stop=True,
                )

        # Explicit global emission order.  Per-engine instruction order equals
        # program order, so this linearization pins every engine's queue:
        # DVE: pkt0,pkt1,pq00,pq01,pkt2,par0,pq10,... (never waits long),
        # PE: warm + conv halves back-to-back with sels trailing ~2 halves,
        # DMA: xf stream paced against conv, xk/qt/kk trickled between.
        peq_tiles = {}

        def new_peq(pair):
            peq_tiles[pair] = cp.tile(
                [128, NT, 128], F16, tag="peq", name=f"peq_{pair}"
            )

        def dma(t, src):
            nc.sync.dma_start(t[:], src)

        def q_done(pair, h2):
            qhat_mul(pair, peq_tiles[pair], h2)

        tree_khat(0, 0, 18)
        new_peq(0)
        conv_half(0, peq_tiles[0], 0)
        tree_khat(0, 18, NKT)
        dma(kk_t, kk_d[:])
        conv_half(0, peq_tiles[0], 1)
        load_xf(2)
        tree_khat(1)
        new_peq(1)
        conv_half(1, peq_tiles[1], 0)
        dma(qt_t, qt_d[:])
        q_done(0, 0)
        conv_half(1, peq_tiles[1], 1)
        load_xk(0)
        tree_khat(2)
        q_done(0, 1)
        load_xf(3)
        new_peq(2)
        conv_half(2, peq_tiles[2], 0)
        q_done(1, 0)
        conv_half(2, peq_tiles[2], 1)
        parity_mul(0)
        q_done(1, 1)
        load_xk(1)
        tree_khat(3)
        load_xf(4)
        new_peq(3)
        conv_half(3, peq_tiles[3], 0)
        qhat_sel(0, 0)
        qhat_sel(0, 1)
        q_done(2, 0)
        conv_half(3, peq_tiles[3], 1)
        parity_mul(1)
        q_done(2, 1)
        load_xk(2)
        load_xf(5)
        new_peq(4)
        conv_half(4, peq_tiles[4], 0)
        qhat_sel(1, 0)
        qhat_sel(1, 1)
        parity_sel(0)
        q_done(3, 0)
        conv_half(4, peq_tiles[4], 1)
        parity_mul(2)
        q_done(3, 1)
        load_xk(3)
        load_xf(6)
        new_peq(5)
        conv_half(5, peq_tiles[5], 0)
        qhat_sel(2, 0)
        qhat_sel(2, 1)
        parity_sel(1)
        q_done(4, 0)
        conv_half(5, peq_tiles[5], 1)
        parity_mul(3)
        q_done(4, 1)
        load_xf(7)
        load_xk(4)
        new_peq(6)
        conv_half(6, peq_tiles[6], 0)
        qhat_sel(3, 0)
        qhat_sel(3, 1)
        parity_sel(2)
        q_done(5, 0)
        conv_half(6, peq_tiles[6], 1)
        parity_mul(4)
        q_done(5, 1)

        load_xk(5)
        load_xk(6)
        load_xk(7)
        new_peq(7)
        conv_half(7, peq_tiles[7], 0)
        qhat_sel(4, 0)
        qhat_sel(4, 1)
        parity_sel(3)
        q_done(6, 0)
        conv_half(7, peq_tiles[7], 1, keep_psum=True)
        parity_mul(5)
        q_done(6, 1)
        # stage-1 drain: qhat pairs 0-4 fully selected
        nc.scalar.copy(qos[:, 0:5, :, :], qps[:, 0:5, :, :])
        qhat_sel(5, 0)
        qhat_sel(5, 1)
        parity_sel(4)
        parity_sel(5)
        nc.sync.dma_start(qo_d[:, 0:5, :, :], qos[:, 0:5, :, :])
        parity_mul(6)
        qhat_sel(6, 0)
        qhat_sel(6, 1)
        parity_sel(6)
        # stage-2 drain: khat parity rows 0-5
        nc.scalar.copy(kos[:, 0:6, :, :], kps[:, 0:6, :, :])
        nc.sync.dma_start(ko_d[:, 0:6, :, :], kos[:, 0:6, :, :])
        qhat_mul_direct(7, 1)
        parity_mul(7)
        qhat_sel(7, 1)
        parity_sel(7)
        q_done(7, 0)
        qhat_sel(7, 0)
        nc.sync.dma_start(kt_d[:], kacc[:])
        nc.scalar.copy(kos[:, 6:N_PAR, :, :], kps[:, 6:N_PAR, :, :])
        nc.sync.dma_start(ko_d[:, 6:N_PAR, :, :], kos[:, 6:N_PAR, :, :])
        nc.scalar.copy(qos[:, 5:PAIRS, :, :], qps[:, 5:PAIRS, :, :])
        nc.sync.dma_start(qo_d[:, 5:PAIRS, :, :], qos[:, 5:PAIRS, :, :])

    nc.compile()
    return nc


def _get_module():
    if "nc" not in _CACHE:
        _CACHE["nc"] = build_module()
    return _CACHE["nc"]


def make_in_maps(queries, keys, noise, conv_weight, num):
    """Host-side shard + re-layout (all cheap numpy ops)."""
    num = int(np.asarray(num))
    queries = np.asarray(queries, dtype=np.float32)
    keys = np.asarray(keys, dtype=np.float32)
    noise = np.asarray(noise, dtype=np.float32)
    w = np.asarray(conv_weight, dtype=np.float32)[0, 0, :]
    scale = 1.0 / math.sqrt(num * D)

    # Toeplitz moving tiles (no scale): mv[s][p, tau] = w[128s + p - tau]
    p = np.arange(128)[:, None]
    tau = np.arange(128)[None, :]
    mv = np.zeros((3, 128, 128), np.float32)
    for s in range(3):
        j = 128 * s + p - tau
        mask = (j >= 0) & (j < K)
        mv[s][mask] = w[j[mask]]
    cst = np.zeros((128, 3 * 128 + 2), np.float16)
    cst[:, 0:384] = mv.transpose(1, 0, 2).reshape(128, 384).astype(np.float16)
    cst[0:64, 384] = 1.0
    cst[64:128, 385] = 1.0

    # conv stationary: xf[pair][p, n, r2, d] = x[2pair+r2][d, 128n+p]
    xf = (
        noise[:, :, : NW * 128]
        .reshape(B * NUM // 2, 2, D, NW, 128)
        .transpose(0, 4, 3, 1, 2)
        .astype(np.float16)
    )  # [64 global pairs, 128, NW, 2, D]

    # queries in (r2,d)-partition layout, scale folded, duplicated over r2:
    # qt[b][64*r2+d, blk, tau] = q[b, d, 128blk+tau] * scale
    qt1 = (queries * scale).reshape(B, D, NT, 128)
    qt = np.empty((B, 128, NT, 128), np.float32)
    qt[:, 0:64] = qt1
    qt[:, 64:128] = qt1
    qt16 = qt.astype(np.float16)

    # conv-layout shifted keys for tree rows:
    # kf[b][p, n, d] = k[b, d, 128n+p-100] * scale (0 OOB)
    kpad = np.zeros((B, D, NKT * 128), np.float32)
    kpad[:, :, K // 2 : K // 2 + L] = keys * scale
    kf16 = kpad.reshape(B, D, NKT, 128).transpose(0, 3, 2, 1).astype(np.float16)

    # parity khat layouts: xk[row][64*par+d, c, i] = x[d, 2*(128c+i)+par+100]
    # kk[b][64*par+d, c, i] = k[b, d, 2*(128c+i)+par] * scale
    xk_all = np.empty((B * NUM, 128, UC * 128), np.float32)
    xk_all[:, 0:64, :] = noise[:, :, 100 : 100 + L : 2]
    xk_all[:, 64:128, :] = noise[:, :, 101 : 101 + L : 2]
    xk16 = xk_all.reshape(B * NUM, 128, UC, 128).astype(np.float16)
    kk = np.empty((B, 128, UC * 128), np.float32)
    ks = keys * scale
    kk[:, 0:64, :] = ks[:, :, 0:L:2]
    kk[:, 64:128, :] = ks[:, :, 1:L:2]
    kk16 = kk.reshape(B, 128, UC, 128).astype(np.float16)

    in_maps = []
    for c in range(N_CORES):
        b = c // 2
        r0 = ROWS * c
        in_maps.append(
            {
                "xf": np.ascontiguousarray(xf[r0 // 2 : r0 // 2 + PAIRS]),
                "cst": cst,
                "qt": np.ascontiguousarray(qt16[b]),
                "kf": np.ascontiguousarray(kf16[b]),
                "xk": np.ascontiguousarray(xk16[r0 + ROWS - N_PAR : r0 + ROWS]),
                "kk": np.ascontiguousarray(kk16[b]),
            }
        )
    return in_maps


def assemble_outputs(results):
    qhat = np.empty((B * NUM, L), np.float32)
    khat = np.empty((B * NUM, L), np.float32)
    for c in range(N_CORES):
        r0 = ROWS * c
        qo = results[c]["qo"]  # [128, PAIRS, NT, 2]
        ko = results[c]["ko"]  # [128, N_PAR, UC, 2]
        kt = results[c]["kt"]  # [128, TPAIRS, NKT, 2]
        qhat[r0 : r0 + ROWS] = qo.transpose(1, 3, 2, 0).reshape(ROWS, L)
        khat[r0 + ROWS - N_PAR : r0 + ROWS] = ko.transpose(1, 2, 0, 3).reshape(
            N_PAR, L
        )
        kv = kt.transpose(1, 3, 2, 0).reshape(2 * TPAIRS, NKT * 128)
        khat[r0 : r0 + 2 * TPAIRS] = kv[:, K // 2 : K // 2 + L]
    return (
        qhat.reshape(B, NUM, L),
        khat.reshape(B, NUM, L),
    )


def kernel(queries, keys, noise, conv_weight, num):
    _ensure_paths()
    from concourse import bass_utils

    in_maps = make_in_maps(queries, keys, noise, conv_weight, num)
    nc = _get_module()
    res = bass_utils.run_bass_kernel_spmd(nc, in_maps, core_ids=list(range(N_CORES)))
    return assemble_outputs(res.results)
